# revision 1
# baseline (speedup 1.0000x reference)
"""ConvTranspose2d(256->128, k=4, stride=2, pad=1) on 8 Trainium2 cores.

Full inputs:  x (16, 256, 64, 64) f32, weight (256, 128, 4, 4) f32, bias (128,) f32
Full output:  (16, 128, 128, 128) f32

Strategy
--------
Data-parallel over batch: each of the 8 cores handles 2 images.

The stride-2 transposed conv decomposes exactly into 4 output parity
classes (ph, pw) in {0,1}^2; each class output pixel (2m+ph, 2n+pw) is a
sum over 4 kernel taps of a 1x1 conv (Cin=256 -> Cout=128 matmul) applied
to a +-1-shifted input pixel:

    ph=0: (kh=1, dh=0), (kh=3, dh=-1)      ph=1: (kh=0, dh=+1), (kh=2, dh=0)
    (same table for pw/kw/dw)

Per core every class-tap is a matmul with K=256 (2 chunks of 128
partitions), M=Cout=128, N = spatial positions.  The moving operand is a
shifted window into a zero-padded SBUF image (66x66 per image per cin
chunk); weights are stationary 128x128 tiles.  Matmuls run as float32r
(fp32 bits, ~FP22 multiply) which streams 1 row/cycle for N>=256, 4x
faster than true fp32.  PSUM accumulates the 8 contributions
(4 taps x 2 cin chunks) per 512-position bank; drains (split across DVE
and ACT) add bias as a per-partition scalar while interleaving the two
column-parity classes into full output rows so the store DMA moves
512B-contiguous segments.  Input rows and weight groups are DMA'd in
first-use order on two parallel HWDGE rings so the PE starts ~3.5 us in.
"""

import sys

sys.path.insert(0, "/opt/trn_rl_repo")

import numpy as np

import concourse.tile as tile
from concourse import bacc, mybir

F32 = mybir.dt.float32
F32R = mybir.dt.float32r

N_CORES = 8
IMGS_PER_CORE = 2
CIN, COUT, KH, KW = 256, 128, 4, 4
H = W = 64
OH = OW = 128
PAD_H = H + 2  # rows -1..64
PAD_W = W + 2
IMG_PAD = PAD_H * PAD_W  # 4356

# taps[parity] = list of (k, shift) pairs contributing to that output parity
TAPS = {0: ((1, 0), (3, -1)), 1: ((0, 1), (2, 0))}

M_BLOCK = 16  # output row-pairs per iteration (16 rows of m -> 32 output rows)


def _build_program(hw_reps=None, store_split=True, psum_per_pw=True, warmup=0,
                   fine_tail=True):
    """Build the single-core Bass program (same program runs on all 8 cores).

    hw_reps: if set, wrap the compute+store section in a hardware For_i loop
    that repeats it hw_reps times (identical, idempotent work) — used only for
    benchmarking marginal per-body execution time.
    """
    nc = bacc.Bacc(
        "TRN2", target_bir_lowering=False, debug=False, num_devices=N_CORES
    )
    x_d = nc.dram_tensor(
        "x", [IMGS_PER_CORE, CIN, PAD_H, PAD_W], F32R, kind="ExternalInput"
    ).ap()
    w_d = nc.dram_tensor("w", [128, 32 * 128], F32R, kind="ExternalInput").ap()
    b_d = nc.dram_tensor("b", [128, 1], F32, kind="ExternalInput").ap()
    out_d = nc.dram_tensor(
        "out", [IMGS_PER_CORE, COUT, OH, OW], F32, kind="ExternalOutput"
    ).ap()

    with tile.TileContext(nc) as tc:
        with (
            tc.tile_pool(name="consts", bufs=1) as consts,
            tc.tile_pool(name="rbp", bufs=3) as rbp,
            tc.tile_pool(name="psp", bufs=4 if psum_per_pw else 2, space="PSUM")
            as psp,
        ):
            w_sb = consts.tile([128, 32 * 128], F32R)
            bias_sb = consts.tile([128, 1], F32)
            x_pad = consts.tile([128, 2 * IMGS_PER_CORE * IMG_PAD], F32R)
            warm_sb = (
                consts.tile([128, 160], F32, name="warm_sb") if warmup else None
            )
            xp = x_pad.rearrange(
                "p (c i r w) -> p c i r w", c=2, i=IMGS_PER_CORE, r=PAD_H, w=PAD_W
            )
            # out viewed as [img, cout, m, ph, w] so step-2 row stores are a slice
            out_v = out_d.rearrange("i co (m two) w -> i co m two w", two=2)

            import contextlib

            rep_ctx = (
                tc.For_i(0, hw_reps, 1) if hw_reps else contextlib.nullcontext()
            )
            with rep_ctx:
                if warmup:
                    # dep-free fp32 matmuls fill the initial DMA-wait window
                    # and warm the PE HAM clock gate (cold 1.2 -> 2.4 GHz
                    # needs ~3.4us of sustained PE activity) so the real
                    # matmul stream starts at full clock
                    nc.vector.memset(warm_sb, 0.0)
                    warm_ps = psp.tile([128, 512], F32, tag="ps")
                    for _ in range(warmup):
                        nc.tensor.matmul(
                            warm_ps[:, 0:32],
                            warm_sb[:, 0:128],
                            warm_sb[:, 128:160],
                            start=True,
                            stop=True,
                        )
                _emit_loads(nc, xp, w_sb, bias_sb, x_d, w_d, b_d)
                _emit_body(
                    nc, xp, w_sb, bias_sb, out_v, psp, rbp, store_split,
                    psum_per_pw, fine_tail,
                )

    nc.compile()
    return nc


def _emit_loads(nc, xp, w_sb, bias_sb, x_d, w_d, b_d):
    # weight tile index t = c*16 + kh*4 + kw.  Load groups in first-use
    # order: the first matmuls are (ph=0 -> kh in {1,3}) with chunk c=0,
    # so (c0,kh1),(c0,kh3) go first, then c1 of the same, then kh in {0,2}.
    # weights go through the scalar engine's HWDGE ring, x through sync's —
    # two independent FIFO rings drain in parallel at startup
    def load_w_group(c, kh):
        t0 = c * 16 + kh * 4
        nc.scalar.dma_start(
            out=w_sb[:, t0 * 128 : (t0 + 4) * 128],
            in_=w_d[:, t0 * 128 : (t0 + 4) * 128],
        )

    # x (host pre-padded to 66x66, zero borders) lands in row-slabs ordered
    # by first use, so the first m-blocks' matmuls start before the whole
    # input has arrived
    SLABS = ((0, 18), (18, 34), (34, 50), (50, PAD_H))

    def load_x_slab(i, s, c):
        lo, hi = SLABS[s]
        nc.sync.dma_start(
            out=xp[:, c, i, lo:hi, :],
            in_=x_d[i, c * 128 : (c + 1) * 128, lo:hi, :],
        )

    load_w_group(0, 1)
    load_w_group(0, 3)
    load_x_slab(0, 0, 0)
    load_w_group(1, 1)
    load_w_group(1, 3)
    load_x_slab(0, 0, 1)
    nc.scalar.dma_start(out=bias_sb, in_=b_d)
    for s in (1, 2, 3):
        for c in range(2):
            load_x_slab(0, s, c)
    for c in range(2):  # kh in {0,2}: first needed at (img0, ph=1)
        load_w_group(c, 0)
        load_w_group(c, 2)
    for s in range(4):
        for c in range(2):
            load_x_slab(1, s, c)


def _emit_body(
    nc, xp, w_sb, bias_sb, out_v, psp, rbp, store_split=True, psum_per_pw=False,
    fine_tail=False,
):
    if True:  # keep indentation structure simple
        if True:
            for img in range(IMGS_PER_CORE):
                for ph in range(2):
                    for m0 in range(0, H, M_BLOCK):
                        # 4 PSUM banks: (pw, half) -> 8 rows x 64 cols each
                        if psum_per_pw:
                            ps_pw = [
                                psp.tile([128, 2 * 512], F32, name=f"ps{pw}", tag="ps")
                                for pw in range(2)
                            ]
                        else:
                            ps = psp.tile([128, 4 * 512], F32)
                            ps_pw = [ps[:, 0:1024], ps[:, 1024:2048]]
                        for pw in range(2):
                            # chunk c outermost: the first matmuls only need
                            # chunk-0 data/weights (earlier pipeline start)
                            tap_list = [
                                (kh, dh, kw, dw, c)
                                for c in range(2)
                                for kh, dh in TAPS[ph]
                                for kw, dw in TAPS[pw]
                            ]
                            for ti, (kh, dh, kw, dw, c) in enumerate(tap_list):
                                t = c * 16 + kh * 4 + kw
                                lhsT = w_sb[:, t * 128 : (t + 1) * 128]
                                for half in range(2):
                                    r0 = 1 + m0 + 8 * half + dh
                                    rhs = xp[
                                        :, c, img, r0 : r0 + 8, 1 + dw : 1 + dw + W
                                    ]
                                    nc.tensor.matmul(
                                        ps_pw[pw][:, half * 512 : (half + 1) * 512],
                                        lhsT,
                                        rhs,
                                        start=(ti == 0),
                                        stop=(ti == len(tap_list) - 1),
                                    )

                        # drain: bias add + interleave column parities;
                        # split across DVE and ACT so neither engine gates PE
                        is_last = (
                            fine_tail
                            and img == IMGS_PER_CORE - 1
                            and ph == 1
                            and m0 == H - M_BLOCK
                        )
                        rb = rbp.tile([128, M_BLOCK * OW], F32)
                        rbv = rb.rearrange("p (m n two) -> p m n two", n=W, two=2)
                        for pw in range(2):
                            # rows 0-7 (half 0) on ACT
                            src = ps_pw[pw][:, 0:512].rearrange(
                                "p (m n) -> p m n", n=W
                            )
                            nc.scalar.activation(
                                rbv[:, 0:8, :, pw],
                                src,
                                func=mybir.ActivationFunctionType.Identity,
                                bias=bias_sb[:, 0:1],
                                scale=1.0,
                            )
                        if not is_last:
                            for pw in range(2):
                                # rows 8-15 (half 1) on the faster DVE so the
                                # final store is gated on the quicker engine
                                src = ps_pw[pw][:, 512:1024].rearrange(
                                    "p (m n) -> p m n", n=W
                                )
                                nc.vector.tensor_scalar_add(
                                    rbv[:, 8:16, :, pw], src, bias_sb[:, 0:1]
                                )
                        else:
                            # last iteration: drain half 1 in 4-row quarters,
                            # top quarter first and pw split across DVE/ACT,
                            # so the last stores are small and launch early -
                            # shortens the kernel tail
                            for q in (1, 0):
                                for pw in range(2):
                                    src = ps_pw[pw][
                                        :, 512 + q * 256 : 768 + q * 256
                                    ].rearrange("p (m n) -> p m n", n=W)
                                    dst = rbv[:, 8 + 4 * q : 12 + 4 * q, :, pw]
                                    if pw == 0:
                                        nc.vector.tensor_scalar_add(
                                            dst, src, bias_sb[:, 0:1]
                                        )
                                    else:
                                        nc.scalar.activation(
                                            dst,
                                            src,
                                            func=mybir.ActivationFunctionType.Identity,
                                            bias=bias_sb[:, 0:1],
                                            scale=1.0,
                                        )

                        # store in halves: each gated only on its own drains,
                        # shortening the end-of-kernel tail.  The very last
                        # iteration stores the top half in 4-row quarters
                        # (top-most first) so the final transfer is small.
                        rbm = rb.rearrange("p (m w) -> p m w", w=OW)
                        if is_last:
                            pieces = ((0, 8), (12, 16), (8, 12))
                        elif store_split:
                            pieces = ((0, 8), (8, 16))
                        else:
                            pieces = ((0, 16),)
                        for lo, hi in pieces:
                            nc.sync.dma_start(
                                out=out_v[img, :, m0 + lo : m0 + hi, ph, :],
                                in_=rbm[:, lo:hi, :],
                            )


_NC_CACHE = {}


def _get_nc():
    if "nc" not in _NC_CACHE:
        _NC_CACHE["nc"] = _build_program()
    return _NC_CACHE["nc"]


def _prep_inputs(x, weight, bias):
    # w[p, (chunk*16 + kh*4 + kw)*128 + co] = weight[chunk*128+p, co, kh, kw]
    w = np.ascontiguousarray(
        np.asarray(weight, np.float32)
        .reshape(2, 128, COUT, KH, KW)
        .transpose(1, 0, 3, 4, 2)  # (p, chunk, kh, kw, co)
        .reshape(128, 32 * 128)
    )
    b = np.ascontiguousarray(np.asarray(bias, np.float32).reshape(128, 1))
    x = np.asarray(x, np.float32)
    xpad = np.zeros((x.shape[0], CIN, PAD_H, PAD_W), np.float32)
    xpad[:, :, 1 : 1 + H, 1 : 1 + W] = x
    return [
        {
            "x": np.ascontiguousarray(
                xpad[i * IMGS_PER_CORE : (i + 1) * IMGS_PER_CORE]
            ),
            "w": w,
            "b": b,
        }
        for i in range(N_CORES)
    ]


def kernel(x, weight, bias):
    from concourse.bass_utils import run_bass_kernel_spmd

    nc = _get_nc()
    in_maps = _prep_inputs(x, weight, bias)
    res = run_bass_kernel_spmd(nc, in_maps, list(range(N_CORES)))
    _NC_CACHE["last_results"] = res
    out = np.concatenate([res.results[i]["out"] for i in range(N_CORES)], axis=0)
    return out.astype(np.float32, copy=False)

